# revision 38
# baseline (speedup 1.0000x reference)
"""BiCGSTAB (4 fixed iterations, 7-point stencil) on 8 Trainium2 NeuronCores.

Problem: x,b,ref: [2,256,256,256] f32, center: [1,256,256,1] f32.
reference() runs 4 BiCGSTAB iterations with A = 7-point stencil
  S(u)[b,h,w,z] = center[h,w]*u - u[w-1] - u[w+1] - u[h-1] - u[h+1] - u[z-1] - u[z+1]
zero Dirichlet boundaries, per-batch dot products.

Sharding: core c handles batch c//4, H-slab [64*(c%4), 64*(c%4)+64).
Dots -> 4-rank AllReduce in groups [[0..3],[4..7]]; H-halos -> AllGather +
indirect-DMA ghost gathers (edge cores index a zero row range).

v2 design (vs the phase-per-field baseline):
- DRAM fields live as [wc][128(w), HC*Z] so every DMA is a clean 2D pattern
  with multi-KB contiguous lines per partition (no transposed descriptors).
- phase B computes s = r - alpha*v on the fly from r,v windows, applies the
  stencil t = S(s), takes the three dots, and stores s,t; phase C is then a
  pure 5-op update pass (x,r,p) on plain blocks - no windows, no stencil
  recompute. HBM traffic/iter drops ~2.3x vs the baseline.
- v, r, p, s, t and halos are stored bf16 (r0, x, b stay f32; all arithmetic
  and dots in f32). Host-side simulation of this exact rounding gives
  rel_err 7.5e-3 (measured on HW: 7.4e-3) vs the 2e-2 gate.
- engine split: all elementwise on DVE (Pool elementwise and non-SP DMA
  issue measured SLOWER on HW), cen-mul + <t,t> square on ACT, W-shifts as
  bf16 matmuls on PE, partition reduces/broadcasts + ghost gathers on Pool.
- at it 0, p == r == r0: windows/blocks of r0 are reused for all three roles.
"""
import numpy as np

import concourse.bacc as bacc
import concourse.bass as bass
import concourse.bass_isa as bass_isa
import concourse.mybir as mybir
import concourse.tile as tile

F32 = mybir.dt.float32
BF16 = mybir.dt.bfloat16
I32 = mybir.dt.int32

N_CORES = 8
GROUP = 4
EPS = 1e-6


def build_program(HC=64, W=256, Z=256, KH=4, ITERS=4, collectives=True,
                  twin_reps=0, bf16=True):
    """Build the per-core SPMD Bass program. HC = H planes per core."""
    assert W == 256 and KH * (HC // KH) == HC
    NB = HC // KH
    CHK = 512                      # psum chunk (free elems)
    NCH = KH * Z // CHK
    SD = BF16 if bf16 else F32     # storage dtype for v, r, p, halos
    RG = [list(range(GROUP)), list(range(GROUP, 2 * GROUP))]
    ZR = GROUP * 2 * W             # zero-row base in halo_out buffers
    MM = mybir.AluOpType.mult
    AD = mybir.AluOpType.add
    SU = mybir.AluOpType.subtract

    twin = twin_reps > 0
    assert not (twin and collectives)
    nc = bacc.Bacc("TRN2", target_bir_lowering=False, debug=False,
                   num_devices=N_CORES)

    ekw = {} if twin else {"kind": "ExternalInput"}
    x_in = [nc.dram_tensor(f"xc{w}", [128, HC * Z], F32, **ekw)
            for w in range(2)]
    b_in = [nc.dram_tensor(f"bc{w}", [128, HC * Z], F32, **ekw)
            for w in range(2)]
    if twin:
        x_out = [nc.dram_tensor(f"xo{w}", [128, HC * Z], F32)
                 for w in range(2)]
        dummy_out = nc.dram_tensor("dummy_o", [1, 8], F32,
                                   kind="ExternalOutput")
    else:
        x_out = [nc.dram_tensor(f"xo{w}", [128, HC * Z], F32,
                                kind="ExternalOutput") for w in range(2)]
    cen_in = nc.dram_tensor("cen", [2, 128, HC], F32, kind="ExternalInput")
    matsF_in = nc.dram_tensor("matsF", [128, 384], F32, kind="ExternalInput")
    matsS_in = nc.dram_tensor("matsS", [128, 384], SD, kind="ExternalInput")
    idx_in = nc.dram_tensor("idx", [2, 128, 2], I32, kind="ExternalInput")

    with tile.TileContext(nc) as tc:
        with (
            tc.tile_pool(name="sb", bufs=2) as sb,
            tc.tile_pool(name="ps", bufs=8, space="PSUM") as ps,
            tc.tile_pool(name="dr", bufs=1, space="DRAM") as dr,
        ):
            _cnt = [0]

            def _nm(pfx):
                _cnt[0] += 1
                return f"{pfx}{_cnt[0]}"

            # ---- persistent DRAM intermediates --------------------------
            r0f = [dr.tile([128, HC * Z], F32, tag=f"r0f{w}", name=f"r0f{w}")
                   for w in range(2)]
            fv = [dr.tile([128, HC * Z], SD, tag=f"fv{w}", name=f"fv{w}")
                  for w in range(2)]
            fr = [dr.tile([128, HC * Z], SD, tag=f"fr{w}", name=f"fr{w}")
                  for w in range(2)]
            fp = [dr.tile([128, HC * Z], SD, tag=f"fp{w}", name=f"fp{w}")
                  for w in range(2)]
            xw = [dr.tile([128, HC * Z], F32, tag=f"xw{w}", name=f"xw{w}")
                  for w in range(2)]
            fs = [dr.tile([128, HC * Z], SD, tag=f"fs{w}", name=f"fs{w}")
                  for w in range(2)]
            ft = [dr.tile([128, HC * Z], SD, tag=f"ft{w}", name=f"ft{w}")
                  for w in range(2)]
            halo_inF = dr.tile([2 * W, Z], F32, tag="hinF")
            halo_outF = dr.tile([ZR + 128, Z], F32, tag="houtF")
            halo_inS = dr.tile([2 * W, Z], SD, tag="hinS")
            halo_outS = dr.tile([ZR + 128, Z], SD, tag="houtS")
            din = dr.tile([1, 8], F32, tag="din")
            dout = dr.tile([1, 8], F32, tag="dout")

            # ---- persistent SBUF constants ------------------------------
            cen_sb = []
            for wc in range(2):
                c = sb.tile([128, HC], F32, tag=f"cen{wc}", bufs=1)
                nc.sync.dma_start(out=c[:], in_=cen_in[wc, :, :])
                cen_sb.append(c)
            matsF = sb.tile([128, 384], F32, tag="matsF", bufs=1)
            nc.sync.dma_start(out=matsF[:], in_=matsF_in[:, :])
            matsS = sb.tile([128, 384], SD, tag="matsS", bufs=1)
            nc.sync.dma_start(out=matsS[:], in_=matsS_in[:, :])
            idx_sb = []
            for wc in range(2):
                it_ = sb.tile([128, 2], I32, tag=f"idx{wc}", bufs=1)
                nc.sync.dma_start(out=it_[:], in_=idx_in[wc, :, :])
                idx_sb.append(it_)

            # zero ghost tails + din
            ztF = sb.tile([128, Z], F32, tag="ghF")
            nc.vector.memset(ztF[:], 0.0)
            nc.sync.dma_start(out=halo_outF[ZR:ZR + 128, :], in_=ztF[:])
            ztS = sb.tile([128, Z], SD, tag="ghS")
            nc.vector.memset(ztS[:], 0.0)
            nc.sync.dma_start(out=halo_outS[ZR:ZR + 128, :], in_=ztS[:])
            z8 = sb.tile([1, 8], F32, tag="z8", bufs=1)
            nc.vector.memset(z8[:], 0.0)
            nc.sync.dma_start(out=din[:, :], in_=z8[:])

            # ---- helpers ------------------------------------------------
            def flat(t):
                return t[:].rearrange("p h z -> p (h z)")

            def stage_plane(src_plane, side, wc, halo_in):
                r_ = side * W + wc * 128
                nc.sync.dma_start(out=halo_in[r_:r_ + 128, :], in_=src_plane)

            def allgather(halo_in, halo_out):
                if collectives:
                    nc.gpsimd.collective_compute(
                        "AllGather", mybir.AluOpType.bypass, replica_groups=RG,
                        ins=[halo_in[:, :].opt()],
                        outs=[halo_out[0:ZR, :].opt()])

            def allreduce():
                if collectives:
                    nc.gpsimd.collective_compute(
                        "AllReduce", AD, replica_groups=RG,
                        ins=[din[:, :].opt()], outs=[dout[:, :].opt()])
                dsb = sb.tile([1, 8], F32, tag="dsb", bufs=6, name=_nm("dsb"))
                nc.sync.dma_start(out=dsb[:], in_=dout[:, :])
                return dsb

            def ghost_gather(dst_plane, wc, side, halo_out):
                nc.gpsimd.indirect_dma_start(
                    out=dst_plane, out_offset=None, in_=halo_out[:, :],
                    in_offset=bass.IndirectOffsetOnAxis(
                        ap=idx_sb[wc][:, side:side + 1], axis=0))

            def load_window(field, wc, j, tag, dt, halo_out=None):
                """[128, KH+2, Z] window of planes j*KH-1 .. j*KH+KH."""
                h0 = j * KH
                win = sb.tile([128, KH + 2, Z], dt, tag=tag,
                              bufs=1 if dt is F32 else 2, name=_nm("win"))
                a = 0 if j == 0 else h0 - 1
                bnd = HC if j == NB - 1 else h0 + KH + 1
                po = 1 if j == 0 else 0
                nc.sync.dma_start(
                    out=flat(win)[:, po * Z:(po + bnd - a) * Z],
                    in_=field[wc][:, a * Z:bnd * Z])
                if halo_out is not None:
                    if j == 0:
                        ghost_gather(win[:, 0, :], wc, 0, halo_out)
                    if j == NB - 1:
                        ghost_gather(win[:, KH + 1, :], wc, 1, halo_out)
                return win

            def load_blk(field, wc, j, tag, dt, bufs=2):
                t_ = sb.tile([128, KH, Z], dt, tag=tag, bufs=bufs,
                             name=_nm("blk"))
                h0 = j * KH
                nc.sync.dma_start(out=flat(t_),
                                  in_=field[wc][:, h0 * Z:(h0 + KH) * Z])
                return t_

            def store_blk(field, src, wc, j):
                h0 = j * KH
                nc.sync.dma_start(out=field[wc][:, h0 * Z:(h0 + KH) * Z],
                                  in_=flat(src))

            def s_window(rfield, rdt, wc, j, nalpha_bc):
                """s = r - alpha*v on planes j*KH-1..j*KH+KH (ghosts from
                halo_outS). Returns (swin, rwin, vwin)."""
                h0 = j * KH
                rwin = load_window(rfield, wc, j, f"rw{wc}{rdt}", rdt)
                vwin = load_window(fv, wc, j, f"vw{wc}", SD)
                swin = sb.tile([128, KH + 2, Z], SD, tag=f"sw{wc}",
                               name=_nm("sw"))
                a = 0 if j == 0 else h0 - 1
                bnd = HC if j == NB - 1 else h0 + KH + 1
                po = 1 if j == 0 else 0
                sl = slice(po, po + bnd - a)
                nc.vector.scalar_tensor_tensor(
                    out=swin[:, sl, :], in0=vwin[:, sl, :],
                    scalar=nalpha_bc[:], in1=rwin[:, sl, :], op0=MM, op1=AD)
                if j == 0:
                    ghost_gather(swin[:, 0, :], wc, 0, halo_outS)
                if j == NB - 1:
                    ghost_gather(swin[:, KH + 1, :], wc, 1, halo_outS)
                return swin, rwin, vwin

            def stencil_f32(wins, wc, j):
                """f32-window stencil (P0 / it 0): cen-mul on ACT, A-matmul,
                vt = cen*u - t1 - psum. Returns f32 tile."""
                h0 = j * KH
                win = wins[wc]
                other = wins[1 - wc]
                A_ap = matsF[:, 0:128]
                Bm = matsF[:, 128:256] if wc == 0 else matsF[:, 256:384]
                t1 = sb.tile([128, KH, Z], F32, tag=f"t1{wc}", name=_nm("t1"))
                nc.vector.tensor_add(out=t1[:], in0=win[:, 0:KH, :],
                                     in1=win[:, 2:KH + 2, :])
                nc.vector.tensor_add(out=t1[:, :, 1:Z], in0=t1[:, :, 1:Z],
                                     in1=win[:, 1:KH + 1, 0:Z - 1])
                nc.vector.tensor_add(out=t1[:, :, 0:Z - 1],
                                     in0=t1[:, :, 0:Z - 1],
                                     in1=win[:, 1:KH + 1, 1:Z])
                vt = sb.tile([128, KH, Z], F32, tag=f"vt{wc}", name=_nm("vt"))
                for j1 in range(KH):
                    h = h0 + j1
                    nc.scalar.mul(out=vt[:, j1, :], in_=win[:, j1 + 1, :],
                                  mul=cen_sb[wc][:, h:h + 1])
                nc.vector.tensor_tensor(out=vt[:], in0=vt[:], in1=t1[:], op=SU)
                wf = flat(win)
                of = flat(other)
                vf = flat(vt)
                for q in range(NCH):
                    c0, c1 = q * CHK, (q + 1) * CHK
                    pt = ps.tile([128, CHK], F32, tag="pt", name=_nm("pt"))
                    nc.tensor.matmul(out=pt[:], lhsT=A_ap,
                                     rhs=wf[:, Z + c0:Z + c1],
                                     start=True, stop=False)
                    nc.tensor.matmul(out=pt[:], lhsT=Bm,
                                     rhs=of[:, Z + c0:Z + c1],
                                     start=False, stop=True)
                    nc.vector.tensor_tensor(out=vf[:, c0:c1],
                                            in0=vf[:, c0:c1], in1=pt[:],
                                            op=SU)
                return vt

            def stencil_sd(wins, wc, j):
                """bf16-window stencil (chunked A/B matmuls, SD output)."""
                h0 = j * KH
                win = wins[wc]
                other = wins[1 - wc]
                A_ap = matsS[:, 0:128]
                Bm = matsS[:, 128:256] if wc == 0 else matsS[:, 256:384]
                t1 = sb.tile([128, KH, Z], SD, tag=f"t1{wc}", name=_nm("t1"))
                nc.vector.tensor_add(out=t1[:], in0=win[:, 0:KH, :],
                                     in1=win[:, 2:KH + 2, :])
                nc.vector.tensor_add(out=t1[:, :, 1:Z], in0=t1[:, :, 1:Z],
                                     in1=win[:, 1:KH + 1, 0:Z - 1])
                nc.vector.tensor_add(out=t1[:, :, 0:Z - 1],
                                     in0=t1[:, :, 0:Z - 1],
                                     in1=win[:, 1:KH + 1, 1:Z])
                vt = sb.tile([128, KH, Z], F32, tag=f"vt{wc}", name=_nm("vt"))
                for j1 in range(KH):
                    h = h0 + j1
                    nc.scalar.mul(out=vt[:, j1, :], in_=win[:, j1 + 1, :],
                                  mul=cen_sb[wc][:, h:h + 1])
                nc.vector.tensor_tensor(out=vt[:], in0=vt[:], in1=t1[:], op=SU)
                outt = sb.tile([128, KH, Z], SD, tag=f"sd{wc}", name=_nm("vs"))
                wf = flat(win)
                of = flat(other)
                vf = flat(outt)
                vtf = flat(vt)
                for q in range(NCH):
                    c0, c1 = q * CHK, (q + 1) * CHK
                    pt = ps.tile([128, CHK], F32, tag="pt", name=_nm("pt"))
                    nc.tensor.matmul(out=pt[:], lhsT=A_ap,
                                     rhs=wf[:, Z + c0:Z + c1],
                                     start=True, stop=False)
                    nc.tensor.matmul(out=pt[:], lhsT=Bm,
                                     rhs=of[:, Z + c0:Z + c1],
                                     start=False, stop=True)
                    nc.vector.tensor_tensor(out=vf[:, c0:c1],
                                            in0=vtf[:, c0:c1], in1=pt[:],
                                            op=SU)
                return outt

            def dot_acc(eng, acc, col, in0, in1, scr):
                """acc[:,col] = rowsum(in0*in1); scr is scratch (clobbered)."""
                eng.scalar_tensor_tensor(
                    out=scr, in0=in0, scalar=1.0, in1=in1, op0=MM, op1=MM,
                    accum_out=acc[:, col:col + 1])

            def new_acc(tag):
                return sb.tile([128, 2 * NB], F32, tag=tag, bufs=2,
                               name=_nm("acc"))

            def finish_dot(acc, col):
                red = sb.tile([128, 1], F32, tag="red", bufs=4, name=_nm("red"))
                nc.vector.tensor_reduce(out=red[:], in_=acc[:],
                                        axis=mybir.AxisListType.X,
                                        op=AD)
                par = sb.tile([128, 1], F32, tag="par", bufs=4, name=_nm("par"))
                nc.gpsimd.partition_all_reduce(par[:], red[:], channels=128,
                                               reduce_op=bass_isa.ReduceOp.add)
                nc.sync.dma_start(out=din[0:1, col:col + 1], in_=par[0:1, 0:1])

            def s_tile():
                return sb.tile([1, 1], F32, tag="dsc", bufs=16, name=_nm("sc"))

            def s_recip_eps(a_ap):
                t_ = s_tile()
                nc.vector.tensor_scalar_add(out=t_[:], in0=a_ap, scalar1=EPS)
                r_ = s_tile()
                nc.vector.reciprocal(out=r_[:], in_=t_[:])
                return r_

            def s_mul(a_ap, b_ap):
                t_ = s_tile()
                nc.vector.tensor_tensor(out=t_[:], in0=a_ap, in1=b_ap, op=MM)
                return t_

            def s_sub(a_ap, b_ap):
                t_ = s_tile()
                nc.vector.tensor_tensor(out=t_[:], in0=a_ap, in1=b_ap, op=SU)
                return t_

            def s_neg(a_ap):
                t_ = s_tile()
                nc.vector.tensor_scalar_mul(out=t_[:], in0=a_ap, scalar1=-1.0)
                return t_

            def bcast(a_ap):
                b_ = sb.tile([128, 1], F32, tag="bc", bufs=8, name=_nm("bc"))
                nc.gpsimd.partition_broadcast(b_[:], a_ap, channels=128)
                return b_

            def stt(eng, out, in0, sc, in1):
                eng.scalar_tensor_tensor(out=out, in0=in0, scalar=sc, in1=in1,
                                         op0=MM, op1=AD)

            def scr_tile(wc):
                return sb.tile([128, KH, Z], F32, tag=f"sc{wc}",
                               name=_nm("scr"))

            # interior blocks first (ghost-dependent edges overlap AllGather)
            ghost_last = list(range(1, NB - 1)) + [0, NB - 1]
            # edge blocks first (their outputs feed the next AllGather)
            ghost_first = [0, NB - 1] + list(range(1, NB - 1))

            from contextlib import ExitStack as _ES
            _loop = _ES()
            if twin:
                _loop.enter_context(tc.For_i(0, twin_reps, 1))

            # ================= P0: r0 = b - S(x); partial rho ============
            for wc in range(2):
                for side, h in ((0, 0), (1, HC - 1)):
                    g = sb.tile([128, Z], F32, tag="ghF", name=_nm("gh"))
                    nc.sync.dma_start(out=g[:],
                                      in_=x_in[wc][:, h * Z:(h + 1) * Z])
                    stage_plane(g[:], side, wc, halo_inF)
            allgather(halo_inF, halo_outF)
            accP = new_acc("accA")
            for bi, j in enumerate(ghost_last):
                wins = (load_window(x_in, 0, j, "rw0F", F32, halo_outF),
                        load_window(x_in, 1, j, "rw1F", F32, halo_outF))
                for wc in range(2):
                    vt = stencil_f32(wins, wc, j)
                    bt = load_blk(b_in, wc, j, f"xb{wc}", F32)
                    r0t = scr_tile(wc)
                    nc.vector.tensor_tensor(out=r0t[:], in0=bt[:], in1=vt[:],
                                            op=SU)
                    dot_acc(nc.vector, accP, 2 * bi + wc,
                            r0t[:], r0t[:], vt[:])
                    store_blk(r0f, r0t, wc, j)
                    if j == 0:
                        stage_plane(r0t[:, 0, :], 0, wc, halo_inF)
                    if j == NB - 1:
                        stage_plane(r0t[:, KH - 1, :], 1, wc, halo_inF)
            finish_dot(accP, 3)
            allgather(halo_inF, halo_outF)   # r0 ghosts for A0/B0/C0 windows

            rho_ap = None
            for it in range(ITERS):
                first = it == 0
                last = it == ITERS - 1
                rsrc, rdt = (r0f, F32) if first else (fr, SD)
                halo_p = halo_outF if first else halo_outS

                # ===== A: v = S(p); partial <r0,v> =====
                accA = new_acc("accA")
                for bi, j in enumerate(ghost_last):
                    psrc = r0f if first else fp
                    pdt = F32 if first else SD
                    wins = (load_window(psrc, 0, j, f"rw0{pdt}", pdt, halo_p),
                            load_window(psrc, 1, j, f"rw1{pdt}", pdt, halo_p))
                    for wc in range(2):
                        if first:
                            vtf = stencil_f32(wins, wc, j)
                            vt = sb.tile([128, KH, Z], SD, tag=f"sd{wc}",
                                         name=_nm("vsd"))
                            nc.scalar.copy(out=vt[:], in_=vtf[:])
                            dot_acc(nc.vector, accA, 2 * bi + wc,
                                    wins[wc][:, 1:KH + 1, :], vtf[:],
                                    scr_tile(wc)[:])
                        else:
                            vt = stencil_sd(wins, wc, j)
                            r0c = load_blk(r0f, wc, j, f"xb{wc}", F32)[:]
                            dot_acc(nc.vector, accA, 2 * bi + wc,
                                    r0c, vt[:], scr_tile(wc)[:])
                        store_blk(fv, vt, wc, j)
                finish_dot(accA, 0)
                dsb = allreduce()
                d1_ap = dsb[0:1, 0:1]
                if first:
                    rho_ap = dsb[0:1, 3:4]
                alpha = s_mul(rho_ap, s_recip_eps(d1_ap)[:])
                nalpha_bc = bcast(s_neg(alpha[:])[:])
                alpha_bc = bcast(alpha[:])

                # ===== B: s = r - alpha v (fly); dots of t = S(s) =========
                for wc in range(2):
                    for side, h in ((0, 0), (1, HC - 1)):
                        rp = sb.tile([128, Z], rdt, tag=f"gp{rdt}",
                                     name=_nm("rp"))
                        nc.sync.dma_start(out=rp[:],
                                          in_=rsrc[wc][:, h * Z:(h + 1) * Z])
                        vp = sb.tile([128, Z], SD, tag="ghS", name=_nm("vp"))
                        nc.sync.dma_start(out=vp[:],
                                          in_=fv[wc][:, h * Z:(h + 1) * Z])
                        sp = sb.tile([128, Z], SD, tag="ghS2", name=_nm("sp"))
                        stt(nc.vector, sp[:], vp[:], nalpha_bc[:], rp[:])
                        stage_plane(sp[:], side, wc, halo_inS)
                allgather(halo_inS, halo_outS)
                accTS = new_acc("accA")
                accTT = new_acc("accB")
                accRT = new_acc("accC")
                for bi, j in enumerate(ghost_last):
                    sw0, rw0, _ = s_window(rsrc, rdt, 0, j, nalpha_bc)
                    sw1, rw1, _ = s_window(rsrc, rdt, 1, j, nalpha_bc)
                    wins = (sw0, sw1)
                    rwins = (rw0, rw1)
                    for wc in range(2):
                        vt = stencil_sd(wins, wc, j)  # t (SD)
                        scr = scr_tile(wc)
                        dot_acc(nc.vector, accTS, 2 * bi + wc,
                                vt[:], wins[wc][:, 1:KH + 1, :], scr[:])
                        nc.scalar.activation(
                            out=scr[:], in_=vt[:],
                            func=mybir.ActivationFunctionType.Square,
                            accum_out=accTT[:, 2 * bi + wc:2 * bi + wc + 1])
                        # store s centers (+ t when needed): C consumes them
                        # as plain blocks instead of recomputing the stencil.
                        nc.sync.dma_start(
                            out=fs[wc][:, j * KH * Z:(j + 1) * KH * Z],
                            in_=wins[wc][:, 1:KH + 1, :].rearrange(
                                "p h z -> p (h z)"))
                        if not last:
                            store_blk(ft, vt, wc, j)
                            if first:
                                r0c = rwins[wc][:, 1:KH + 1, :]
                            else:
                                r0c = load_blk(r0f, wc, j, f"xb{wc}", F32)[:]
                            dot_acc(nc.vector, accRT, 2 * bi + wc,
                                    r0c, vt[:], scr[:])
                finish_dot(accTS, 0)
                finish_dot(accTT, 1)
                if not last:
                    finish_dot(accRT, 2)
                dsb = allreduce()
                omega = s_mul(dsb[0:1, 0:1], s_recip_eps(dsb[0:1, 1:2])[:])
                omega_bc = bcast(omega[:])
                nomega_bc = bcast(s_neg(omega[:])[:])
                if not last:
                    rho_n = s_sub(s_sub(rho_ap, s_mul(alpha[:], d1_ap)[:])[:],
                                  s_mul(omega[:], dsb[0:1, 2:3])[:])
                    beta = s_mul(
                        s_mul(rho_n[:], s_recip_eps(rho_ap)[:])[:],
                        s_mul(alpha[:], s_recip_eps(omega[:])[:])[:])
                    beta_bc = bcast(beta[:])
                    rho_ap = rho_n[:]

                # ===== C: pure update pass (s,t loaded, no stencil):
                #          x += alpha p + omega s; r = s - omega t;
                #          p = r + beta (p - omega v) =====================
                xsrc = x_in if first else xw
                xdst = x_out if last else xw
                order = list(range(NB)) if last else ghost_first
                for oi, j in enumerate(order):
                    for wc in range(2):
                        sc_ = load_blk(fs, wc, j, f"sb{wc}", SD)[:]
                        xt = load_blk(xsrc, wc, j, f"xb{wc}", F32)
                        if first:
                            pc = load_blk(r0f, wc, j, f"pbF{wc}", F32,
                                          bufs=1)[:]
                        else:
                            pc = load_blk(fp, wc, j, f"pb{wc}", SD)[:]
                        x1 = scr_tile(wc)
                        stt(nc.vector, x1[:], pc, alpha_bc[:], xt[:])
                        x2 = scr_tile(wc)
                        stt(nc.vector, x2[:], sc_, omega_bc[:], x1[:])
                        store_blk(xdst, x2, wc, j)
                        if not last:
                            tc_ = load_blk(ft, wc, j, f"tb{wc}", SD)[:]
                            vc = load_blk(fv, wc, j, f"vb{wc}", SD)[:]
                            rt = sb.tile([128, KH, Z], SD, tag=f"q{wc}",
                                         bufs=4, name=_nm("rt"))
                            stt(nc.vector, rt[:], tc_, nomega_bc[:], sc_)
                            store_blk(fr, rt, wc, j)
                            u = sb.tile([128, KH, Z], SD, tag=f"q{wc}",
                                        bufs=4, name=_nm("u"))
                            stt(nc.vector, u[:], vc, nomega_bc[:], pc)
                            po_ = sb.tile([128, KH, Z], SD, tag=f"q{wc}",
                                          bufs=4, name=_nm("po"))
                            stt(nc.vector, po_[:], u[:], beta_bc[:], rt[:])
                            store_blk(fp, po_, wc, j)
                            if j == 0:
                                stage_plane(po_[:, 0, :], 0, wc, halo_inS)
                            if j == NB - 1:
                                stage_plane(po_[:, KH - 1, :], 1, wc,
                                            halo_inS)
                    if not last and oi == 1:
                        allgather(halo_inS, halo_outS)

            _loop.close()
            if twin:
                nc.sync.dma_start(out=dummy_out[:, :], in_=z8[:])

    nc.compile()
    return nc


# ---------------------------------------------------------------------------
# host-side wrapper
# ---------------------------------------------------------------------------
_CACHE = {}


def _shift_mats():
    A = np.zeros((128, 128), np.float32)
    for i in range(127):
        A[i, i + 1] = 1.0
        A[i + 1, i] = 1.0
    B01 = np.zeros((128, 128), np.float32)
    B01[0, 127] = 1.0
    B10 = np.zeros((128, 128), np.float32)
    B10[127, 0] = 1.0
    return np.concatenate([A, B01, B10], axis=1)


def make_in_maps(x, b, center, HC, W, Z, bf16=True):
    import ml_dtypes
    BF = ml_dtypes.bfloat16 if bf16 else np.float32
    mats = _shift_mats()
    matsS = mats.astype(BF)
    ZR = GROUP * 2 * W
    in_maps = []
    for c in range(N_CORES):
        bi, s = divmod(c, GROUP)
        h0 = s * HC
        # [HC, W, Z] -> [W, HC, Z] -> [2][128, HC*Z]
        xc = np.ascontiguousarray(
            x[bi, h0:h0 + HC].transpose(1, 0, 2)).reshape(2, 128, HC * Z)
        bc = np.ascontiguousarray(
            b[bi, h0:h0 + HC].transpose(1, 0, 2)).reshape(2, 128, HC * Z)
        cen = np.ascontiguousarray(
            center[0, h0:h0 + HC, :, 0].astype(np.float32).T).reshape(
                2, 128, HC)
        w = np.arange(W, dtype=np.int32)
        lo = (s - 1) * 2 * W + W + w if s > 0 else ZR + (w % 128)
        hi = (s + 1) * 2 * W + w if s < GROUP - 1 else ZR + (w % 128)
        idx = np.stack([lo, hi], axis=1).astype(np.int32).reshape(2, 128, 2)
        in_maps.append({
            "xc0": xc[0], "xc1": xc[1],
            "bc0": bc[0], "bc1": bc[1],
            "cen": cen, "matsF": mats, "matsS": matsS, "idx": idx,
        })
    return in_maps


RUN_WALL_S = []


def kernel(x, b, ref, center):
    """Full inputs in, full output out. ref is unused by the reference."""
    import time as _time
    B, H, W, Z = x.shape
    HC = H // GROUP
    key = (HC, W, Z)
    if key not in _CACHE:
        _CACHE[key] = build_program(HC=HC, W=W, Z=Z)
    nc = _CACHE[key]

    from concourse.bass_utils import run_bass_kernel_spmd
    in_maps = make_in_maps(np.asarray(x), np.asarray(b), np.asarray(center),
                           HC, W, Z)
    _t0 = _time.time()
    res = run_bass_kernel_spmd(nc, in_maps, core_ids=list(range(N_CORES)))
    RUN_WALL_S.append(_time.time() - _t0)
    out = np.empty((B, H, W, Z), np.float32)
    for c in range(N_CORES):
        bi, s = divmod(c, GROUP)
        xo = np.concatenate([res.results[c]["xo0"], res.results[c]["xo1"]],
                            axis=0)  # [256, HC*Z]
        out[bi, s * HC:(s + 1) * HC] = xo.reshape(
            W, HC, Z).transpose(1, 0, 2)
    return out


# revision 43
# speedup vs baseline: 1.0478x; 1.0478x over previous
"""BiCGSTAB (4 fixed iterations, 7-point stencil) on 8 Trainium2 NeuronCores.

Problem: x,b,ref: [2,256,256,256] f32, center: [1,256,256,1] f32.
reference() runs 4 BiCGSTAB iterations with A = 7-point stencil
  S(u)[b,h,w,z] = center[h,w]*u - u[w-1] - u[w+1] - u[h-1] - u[h+1] - u[z-1] - u[z+1]
zero Dirichlet boundaries, per-batch dot products.

Sharding: core c handles batch c//4, H-slab [64*(c%4), 64*(c%4)+64).
Dots -> 4-rank AllReduce in groups [[0..3],[4..7]]; H-halos -> AllGather +
indirect-DMA ghost gathers (edge cores index a zero row range).

v2 design (vs the phase-per-field baseline):
- DRAM fields live as [wc][128(w), HC*Z] so every DMA is a clean 2D pattern
  with multi-KB contiguous lines per partition (no transposed descriptors).
- phase B computes s = r - alpha*v on the fly from r,v windows, applies the
  stencil t = S(s), takes the three dots, and stores s,t; phase C is then a
  pure 5-op update pass (x,r,p) on plain blocks - no windows, no stencil
  recompute. HBM traffic/iter drops ~2.3x vs the baseline.
- v, r, p, s, t and halos are stored bf16 (r0, x, b stay f32; all arithmetic
  and dots in f32). Host-side simulation of this exact rounding gives
  rel_err 7.5e-3 (measured on HW: 7.4e-3) vs the 2e-2 gate.
- engine split: all elementwise on DVE (Pool elementwise and non-SP DMA
  issue measured SLOWER on HW), cen-mul + <t,t> square on ACT, W-shifts as
  bf16 matmuls on PE, partition reduces/broadcasts + ghost gathers on Pool.
- at it 0, p == r == r0: windows/blocks of r0 are reused for all three roles.
"""
import numpy as np

import concourse.bacc as bacc
import concourse.bass as bass
import concourse.bass_isa as bass_isa
import concourse.mybir as mybir
import concourse.tile as tile

F32 = mybir.dt.float32
BF16 = mybir.dt.bfloat16
I32 = mybir.dt.int32

N_CORES = 8
GROUP = 4
EPS = 1e-6


def build_program(HC=64, W=256, Z=256, KH=4, ITERS=4, collectives=True,
                  twin_reps=0, bf16=True):
    """Build the per-core SPMD Bass program. HC = H planes per core."""
    assert W == 256 and KH * (HC // KH) == HC
    NB = HC // KH
    CHK = 512                      # psum chunk (free elems)
    NCH = KH * Z // CHK
    SD = BF16 if bf16 else F32     # storage dtype for v, r, p, halos
    RG = [list(range(GROUP)), list(range(GROUP, 2 * GROUP))]
    ZR = GROUP * 2 * W             # zero-row base in halo_out buffers
    MM = mybir.AluOpType.mult
    AD = mybir.AluOpType.add
    SU = mybir.AluOpType.subtract

    twin = twin_reps > 0
    assert not (twin and collectives)
    nc = bacc.Bacc("TRN2", target_bir_lowering=False, debug=False,
                   num_devices=N_CORES)

    ekw = {} if twin else {"kind": "ExternalInput"}
    x_in = [nc.dram_tensor(f"xc{w}", [128, HC * Z], F32, **ekw)
            for w in range(2)]
    b_in = [nc.dram_tensor(f"bc{w}", [128, HC * Z], F32, **ekw)
            for w in range(2)]
    if twin:
        x_out = [nc.dram_tensor(f"xo{w}", [128, HC * Z], F32)
                 for w in range(2)]
        dummy_out = nc.dram_tensor("dummy_o", [1, 8], F32,
                                   kind="ExternalOutput")
    else:
        x_out = [nc.dram_tensor(f"xo{w}", [128, HC * Z], F32,
                                kind="ExternalOutput") for w in range(2)]
    cen_in = nc.dram_tensor("cen", [2, 128, HC], F32, kind="ExternalInput")
    matsF_in = nc.dram_tensor("matsF", [128, 384], F32, kind="ExternalInput")
    matsS_in = nc.dram_tensor("matsS", [128, 384], SD, kind="ExternalInput")
    idx_in = nc.dram_tensor("idx", [2, 128, 2], I32, kind="ExternalInput")

    with tile.TileContext(nc) as tc:
        with (
            tc.tile_pool(name="sb", bufs=2) as sb,
            tc.tile_pool(name="ps", bufs=8, space="PSUM") as ps,
            tc.tile_pool(name="dr", bufs=1, space="DRAM") as dr,
        ):
            _cnt = [0]

            def _nm(pfx):
                _cnt[0] += 1
                return f"{pfx}{_cnt[0]}"

            # ---- persistent DRAM intermediates --------------------------
            r0f = [dr.tile([128, HC * Z], F32, tag=f"r0f{w}", name=f"r0f{w}")
                   for w in range(2)]
            fv = [dr.tile([128, HC * Z], SD, tag=f"fv{w}", name=f"fv{w}")
                  for w in range(2)]
            fr = [dr.tile([128, HC * Z], SD, tag=f"fr{w}", name=f"fr{w}")
                  for w in range(2)]
            fp = [dr.tile([128, HC * Z], SD, tag=f"fp{w}", name=f"fp{w}")
                  for w in range(2)]
            xw = [dr.tile([128, HC * Z], F32, tag=f"xw{w}", name=f"xw{w}")
                  for w in range(2)]
            fs = [dr.tile([128, HC * Z], SD, tag=f"fs{w}", name=f"fs{w}")
                  for w in range(2)]
            ft = [dr.tile([128, HC * Z], SD, tag=f"ft{w}", name=f"ft{w}")
                  for w in range(2)]
            halo_inF = dr.tile([2 * W, Z], F32, tag="hinF")
            halo_outF = dr.tile([ZR + 128, Z], F32, tag="houtF")
            halo_inS = dr.tile([2 * W, Z], SD, tag="hinS")
            halo_outS = dr.tile([ZR + 128, Z], SD, tag="houtS")
            din = dr.tile([1, 8], F32, tag="din")
            dout = dr.tile([1, 8], F32, tag="dout")

            # ---- persistent SBUF constants ------------------------------
            cen_sb = []
            for wc in range(2):
                c = sb.tile([128, HC], F32, tag=f"cen{wc}", bufs=1)
                nc.sync.dma_start(out=c[:], in_=cen_in[wc, :, :])
                cen_sb.append(c)
            matsF = sb.tile([128, 384], F32, tag="matsF", bufs=1)
            nc.sync.dma_start(out=matsF[:], in_=matsF_in[:, :])
            matsS = sb.tile([128, 384], SD, tag="matsS", bufs=1)
            nc.sync.dma_start(out=matsS[:], in_=matsS_in[:, :])
            idx_sb = []
            for wc in range(2):
                it_ = sb.tile([128, 2], I32, tag=f"idx{wc}", bufs=1)
                nc.sync.dma_start(out=it_[:], in_=idx_in[wc, :, :])
                idx_sb.append(it_)

            # zero ghost tails + din
            ztF = sb.tile([128, Z], F32, tag="ghF")
            nc.vector.memset(ztF[:], 0.0)
            nc.sync.dma_start(out=halo_outF[ZR:ZR + 128, :], in_=ztF[:])
            ztS = sb.tile([128, Z], SD, tag="ghS")
            nc.vector.memset(ztS[:], 0.0)
            nc.sync.dma_start(out=halo_outS[ZR:ZR + 128, :], in_=ztS[:])
            z8 = sb.tile([1, 8], F32, tag="z8", bufs=1)
            nc.vector.memset(z8[:], 0.0)
            nc.sync.dma_start(out=din[:, :], in_=z8[:])

            # ---- helpers ------------------------------------------------
            def flat(t):
                return t[:].rearrange("p h z -> p (h z)")

            def stage_plane(src_plane, side, wc, halo_in):
                r_ = side * W + wc * 128
                nc.sync.dma_start(out=halo_in[r_:r_ + 128, :], in_=src_plane)

            def allgather(halo_in, halo_out):
                if collectives:
                    nc.gpsimd.collective_compute(
                        "AllGather", mybir.AluOpType.bypass, replica_groups=RG,
                        ins=[halo_in[:, :].opt()],
                        outs=[halo_out[0:ZR, :].opt()])

            def allreduce():
                if collectives:
                    nc.gpsimd.collective_compute(
                        "AllReduce", AD, replica_groups=RG,
                        ins=[din[:, :].opt()], outs=[dout[:, :].opt()])
                dsb = sb.tile([1, 8], F32, tag="dsb", bufs=6, name=_nm("dsb"))
                nc.sync.dma_start(out=dsb[:], in_=dout[:, :])
                return dsb

            def ghost_gather(dst_plane, wc, side, halo_out):
                nc.gpsimd.indirect_dma_start(
                    out=dst_plane, out_offset=None, in_=halo_out[:, :],
                    in_offset=bass.IndirectOffsetOnAxis(
                        ap=idx_sb[wc][:, side:side + 1], axis=0))

            def load_window(field, wc, j, tag, dt, halo_out=None):
                """[128, KH+2, Z] window of planes j*KH-1 .. j*KH+KH."""
                h0 = j * KH
                win = sb.tile([128, KH + 2, Z], dt, tag=tag,
                              name=_nm("win"))
                a = 0 if j == 0 else h0 - 1
                bnd = HC if j == NB - 1 else h0 + KH + 1
                po = 1 if j == 0 else 0
                nc.sync.dma_start(
                    out=flat(win)[:, po * Z:(po + bnd - a) * Z],
                    in_=field[wc][:, a * Z:bnd * Z])
                if halo_out is not None:
                    if j == 0:
                        ghost_gather(win[:, 0, :], wc, 0, halo_out)
                    if j == NB - 1:
                        ghost_gather(win[:, KH + 1, :], wc, 1, halo_out)
                return win

            def load_blk(field, wc, j, tag, dt, bufs=2):
                t_ = sb.tile([128, KH, Z], dt, tag=tag, bufs=bufs,
                             name=_nm("blk"))
                h0 = j * KH
                nc.sync.dma_start(out=flat(t_),
                                  in_=field[wc][:, h0 * Z:(h0 + KH) * Z])
                return t_

            def store_blk(field, src, wc, j):
                h0 = j * KH
                nc.sync.dma_start(out=field[wc][:, h0 * Z:(h0 + KH) * Z],
                                  in_=flat(src))

            def s_window(rfield, rdt, wc, j, nalpha_bc):
                """s = r - alpha*v on planes j*KH-1..j*KH+KH (ghosts from
                halo_outS). Returns (swin, rwin, vwin)."""
                h0 = j * KH
                rwin = load_window(rfield, wc, j, f"rw{wc}{rdt}", rdt)
                vwin = load_window(fv, wc, j, f"vw{wc}", SD)
                swin = sb.tile([128, KH + 2, Z], SD, tag=f"sw{wc}",
                               name=_nm("sw"))
                a = 0 if j == 0 else h0 - 1
                bnd = HC if j == NB - 1 else h0 + KH + 1
                po = 1 if j == 0 else 0
                sl = slice(po, po + bnd - a)
                nc.vector.scalar_tensor_tensor(
                    out=swin[:, sl, :], in0=vwin[:, sl, :],
                    scalar=nalpha_bc[:], in1=rwin[:, sl, :], op0=MM, op1=AD)
                if j == 0:
                    ghost_gather(swin[:, 0, :], wc, 0, halo_outS)
                if j == NB - 1:
                    ghost_gather(swin[:, KH + 1, :], wc, 1, halo_outS)
                return swin, rwin, vwin

            def stencil_f32(wins, wc, j):
                """f32-window stencil (P0 / it 0): cen-mul on ACT, A-matmul,
                vt = cen*u - t1 - psum. Returns f32 tile."""
                h0 = j * KH
                win = wins[wc]
                other = wins[1 - wc]
                A_ap = matsF[:, 0:128]
                Bm = matsF[:, 128:256] if wc == 0 else matsF[:, 256:384]
                t1 = sb.tile([128, KH, Z], F32, tag=f"t1{wc}", name=_nm("t1"))
                nc.vector.tensor_add(out=t1[:], in0=win[:, 0:KH, :],
                                     in1=win[:, 2:KH + 2, :])
                nc.vector.tensor_add(out=t1[:, :, 1:Z], in0=t1[:, :, 1:Z],
                                     in1=win[:, 1:KH + 1, 0:Z - 1])
                nc.vector.tensor_add(out=t1[:, :, 0:Z - 1],
                                     in0=t1[:, :, 0:Z - 1],
                                     in1=win[:, 1:KH + 1, 1:Z])
                vt = sb.tile([128, KH, Z], F32, tag=f"vt{wc}", name=_nm("vt"))
                for j1 in range(KH):
                    h = h0 + j1
                    nc.scalar.mul(out=vt[:, j1, :], in_=win[:, j1 + 1, :],
                                  mul=cen_sb[wc][:, h:h + 1])
                nc.vector.tensor_tensor(out=vt[:], in0=vt[:], in1=t1[:], op=SU)
                wf = flat(win)
                of = flat(other)
                vf = flat(vt)
                for q in range(NCH):
                    c0, c1 = q * CHK, (q + 1) * CHK
                    pt = ps.tile([128, CHK], F32, tag="pt", name=_nm("pt"))
                    nc.tensor.matmul(out=pt[:], lhsT=A_ap,
                                     rhs=wf[:, Z + c0:Z + c1],
                                     start=True, stop=False)
                    nc.tensor.matmul(out=pt[:], lhsT=Bm,
                                     rhs=of[:, Z + c0:Z + c1],
                                     start=False, stop=True)
                    nc.vector.tensor_tensor(out=vf[:, c0:c1],
                                            in0=vf[:, c0:c1], in1=pt[:],
                                            op=SU)
                return vt

            def stencil_sd(wins, wc, j):
                """bf16-window stencil (chunked A/B matmuls, SD output)."""
                h0 = j * KH
                win = wins[wc]
                other = wins[1 - wc]
                A_ap = matsS[:, 0:128]
                Bm = matsS[:, 128:256] if wc == 0 else matsS[:, 256:384]
                t1 = sb.tile([128, KH, Z], SD, tag=f"t1{wc}", name=_nm("t1"))
                nc.vector.tensor_add(out=t1[:], in0=win[:, 0:KH, :],
                                     in1=win[:, 2:KH + 2, :])
                nc.vector.tensor_add(out=t1[:, :, 1:Z], in0=t1[:, :, 1:Z],
                                     in1=win[:, 1:KH + 1, 0:Z - 1])
                nc.vector.tensor_add(out=t1[:, :, 0:Z - 1],
                                     in0=t1[:, :, 0:Z - 1],
                                     in1=win[:, 1:KH + 1, 1:Z])
                vt = sb.tile([128, KH, Z], F32, tag=f"vt{wc}", name=_nm("vt"))
                for j1 in range(KH):
                    h = h0 + j1
                    nc.scalar.mul(out=vt[:, j1, :], in_=win[:, j1 + 1, :],
                                  mul=cen_sb[wc][:, h:h + 1])
                nc.vector.tensor_tensor(out=vt[:], in0=vt[:], in1=t1[:], op=SU)
                outt = sb.tile([128, KH, Z], SD, tag=f"sd{wc}", name=_nm("vs"))
                wf = flat(win)
                of = flat(other)
                vf = flat(outt)
                vtf = flat(vt)
                for q in range(NCH):
                    c0, c1 = q * CHK, (q + 1) * CHK
                    pt = ps.tile([128, CHK], F32, tag="pt", name=_nm("pt"))
                    nc.tensor.matmul(out=pt[:], lhsT=A_ap,
                                     rhs=wf[:, Z + c0:Z + c1],
                                     start=True, stop=False)
                    nc.tensor.matmul(out=pt[:], lhsT=Bm,
                                     rhs=of[:, Z + c0:Z + c1],
                                     start=False, stop=True)
                    nc.vector.tensor_tensor(out=vf[:, c0:c1],
                                            in0=vtf[:, c0:c1], in1=pt[:],
                                            op=SU)
                return outt

            def dot_acc(eng, acc, col, in0, in1, scr):
                """acc[:,col] = rowsum(in0*in1); scr is scratch (clobbered)."""
                eng.scalar_tensor_tensor(
                    out=scr, in0=in0, scalar=1.0, in1=in1, op0=MM, op1=MM,
                    accum_out=acc[:, col:col + 1])

            def new_acc(tag):
                return sb.tile([128, 2 * NB], F32, tag=tag, bufs=2,
                               name=_nm("acc"))

            def finish_dot(acc, col):
                red = sb.tile([128, 1], F32, tag="red", bufs=4, name=_nm("red"))
                nc.vector.tensor_reduce(out=red[:], in_=acc[:],
                                        axis=mybir.AxisListType.X,
                                        op=AD)
                par = sb.tile([128, 1], F32, tag="par", bufs=4, name=_nm("par"))
                nc.gpsimd.partition_all_reduce(par[:], red[:], channels=128,
                                               reduce_op=bass_isa.ReduceOp.add)
                nc.sync.dma_start(out=din[0:1, col:col + 1], in_=par[0:1, 0:1])

            def s_tile():
                return sb.tile([1, 1], F32, tag="dsc", bufs=16, name=_nm("sc"))

            def s_recip_eps(a_ap):
                t_ = s_tile()
                nc.vector.tensor_scalar_add(out=t_[:], in0=a_ap, scalar1=EPS)
                r_ = s_tile()
                nc.vector.reciprocal(out=r_[:], in_=t_[:])
                return r_

            def s_mul(a_ap, b_ap):
                t_ = s_tile()
                nc.vector.tensor_tensor(out=t_[:], in0=a_ap, in1=b_ap, op=MM)
                return t_

            def s_sub(a_ap, b_ap):
                t_ = s_tile()
                nc.vector.tensor_tensor(out=t_[:], in0=a_ap, in1=b_ap, op=SU)
                return t_

            def s_neg(a_ap):
                t_ = s_tile()
                nc.vector.tensor_scalar_mul(out=t_[:], in0=a_ap, scalar1=-1.0)
                return t_

            def bcast(a_ap):
                b_ = sb.tile([128, 1], F32, tag="bc", bufs=8, name=_nm("bc"))
                nc.gpsimd.partition_broadcast(b_[:], a_ap, channels=128)
                return b_

            def stt(eng, out, in0, sc, in1):
                eng.scalar_tensor_tensor(out=out, in0=in0, scalar=sc, in1=in1,
                                         op0=MM, op1=AD)

            def scr_tile(wc):
                return sb.tile([128, KH, Z], F32, tag=f"sc{wc}",
                               name=_nm("scr"))

            # interior blocks first (ghost-dependent edges overlap AllGather)
            ghost_last = list(range(1, NB - 1)) + [0, NB - 1]
            # edge blocks first (their outputs feed the next AllGather)
            ghost_first = [0, NB - 1] + list(range(1, NB - 1))

            from contextlib import ExitStack as _ES
            _loop = _ES()
            if twin:
                _loop.enter_context(tc.For_i(0, twin_reps, 1))

            # ================= P0: r0 = b - S(x); partial rho ============
            for wc in range(2):
                for side, h in ((0, 0), (1, HC - 1)):
                    g = sb.tile([128, Z], F32, tag="ghF", name=_nm("gh"))
                    nc.sync.dma_start(out=g[:],
                                      in_=x_in[wc][:, h * Z:(h + 1) * Z])
                    stage_plane(g[:], side, wc, halo_inF)
            allgather(halo_inF, halo_outF)
            accP = new_acc("accA")
            for bi, j in enumerate(ghost_last):
                wins = (load_window(x_in, 0, j, "rw0F", F32, halo_outF),
                        load_window(x_in, 1, j, "rw1F", F32, halo_outF))
                for wc in range(2):
                    vt = stencil_f32(wins, wc, j)
                    bt = load_blk(b_in, wc, j, f"xb{wc}", F32)
                    r0t = scr_tile(wc)
                    nc.vector.tensor_tensor(out=r0t[:], in0=bt[:], in1=vt[:],
                                            op=SU)
                    dot_acc(nc.vector, accP, 2 * bi + wc,
                            r0t[:], r0t[:], vt[:])
                    store_blk(r0f, r0t, wc, j)
                    if j == 0:
                        stage_plane(r0t[:, 0, :], 0, wc, halo_inF)
                    if j == NB - 1:
                        stage_plane(r0t[:, KH - 1, :], 1, wc, halo_inF)
            finish_dot(accP, 3)
            allgather(halo_inF, halo_outF)   # r0 ghosts for A0/B0/C0 windows

            rho_ap = None
            for it in range(ITERS):
                first = it == 0
                last = it == ITERS - 1
                rsrc, rdt = (r0f, F32) if first else (fr, SD)
                halo_p = halo_outF if first else halo_outS

                # ===== A: v = S(p); partial <r0,v> =====
                accA = new_acc("accA")
                for bi, j in enumerate(ghost_last):
                    psrc = r0f if first else fp
                    pdt = F32 if first else SD
                    wins = (load_window(psrc, 0, j, f"rw0{pdt}", pdt, halo_p),
                            load_window(psrc, 1, j, f"rw1{pdt}", pdt, halo_p))
                    for wc in range(2):
                        if first:
                            vtf = stencil_f32(wins, wc, j)
                            vt = sb.tile([128, KH, Z], SD, tag=f"sd{wc}",
                                         name=_nm("vsd"))
                            nc.scalar.copy(out=vt[:], in_=vtf[:])
                            dot_acc(nc.vector, accA, 2 * bi + wc,
                                    wins[wc][:, 1:KH + 1, :], vtf[:],
                                    scr_tile(wc)[:])
                        else:
                            vt = stencil_sd(wins, wc, j)
                            r0c = load_blk(r0f, wc, j, f"xb{wc}", F32)[:]
                            dot_acc(nc.vector, accA, 2 * bi + wc,
                                    r0c, vt[:], scr_tile(wc)[:])
                        store_blk(fv, vt, wc, j)
                finish_dot(accA, 0)
                dsb = allreduce()
                d1_ap = dsb[0:1, 0:1]
                if first:
                    rho_ap = dsb[0:1, 3:4]
                alpha = s_mul(rho_ap, s_recip_eps(d1_ap)[:])
                nalpha_bc = bcast(s_neg(alpha[:])[:])
                alpha_bc = bcast(alpha[:])

                # ===== B: s = r - alpha v (fly); dots of t = S(s) =========
                for wc in range(2):
                    for side, h in ((0, 0), (1, HC - 1)):
                        rp = sb.tile([128, Z], rdt, tag=f"gp{rdt}",
                                     name=_nm("rp"))
                        nc.sync.dma_start(out=rp[:],
                                          in_=rsrc[wc][:, h * Z:(h + 1) * Z])
                        vp = sb.tile([128, Z], SD, tag="ghS", name=_nm("vp"))
                        nc.sync.dma_start(out=vp[:],
                                          in_=fv[wc][:, h * Z:(h + 1) * Z])
                        sp = sb.tile([128, Z], SD, tag="ghS2", name=_nm("sp"))
                        stt(nc.vector, sp[:], vp[:], nalpha_bc[:], rp[:])
                        stage_plane(sp[:], side, wc, halo_inS)
                allgather(halo_inS, halo_outS)
                accTS = new_acc("accA")
                accTT = new_acc("accB")
                accRT = new_acc("accC")
                for bi, j in enumerate(ghost_last):
                    sw0, rw0, _ = s_window(rsrc, rdt, 0, j, nalpha_bc)
                    sw1, rw1, _ = s_window(rsrc, rdt, 1, j, nalpha_bc)
                    wins = (sw0, sw1)
                    rwins = (rw0, rw1)
                    for wc in range(2):
                        vt = stencil_sd(wins, wc, j)  # t (SD)
                        scr = scr_tile(wc)
                        dot_acc(nc.vector, accTS, 2 * bi + wc,
                                vt[:], wins[wc][:, 1:KH + 1, :], scr[:])
                        nc.scalar.activation(
                            out=scr[:], in_=vt[:],
                            func=mybir.ActivationFunctionType.Square,
                            accum_out=accTT[:, 2 * bi + wc:2 * bi + wc + 1])
                        # store s centers (+ t when needed): C consumes them
                        # as plain blocks instead of recomputing the stencil.
                        nc.sync.dma_start(
                            out=fs[wc][:, j * KH * Z:(j + 1) * KH * Z],
                            in_=wins[wc][:, 1:KH + 1, :].rearrange(
                                "p h z -> p (h z)"))
                        if not last:
                            store_blk(ft, vt, wc, j)
                            if first:
                                r0c = rwins[wc][:, 1:KH + 1, :]
                            else:
                                r0c = load_blk(r0f, wc, j, f"xb{wc}", F32)[:]
                            dot_acc(nc.vector, accRT, 2 * bi + wc,
                                    r0c, vt[:], scr[:])
                finish_dot(accTS, 0)
                finish_dot(accTT, 1)
                if not last:
                    finish_dot(accRT, 2)
                dsb = allreduce()
                omega = s_mul(dsb[0:1, 0:1], s_recip_eps(dsb[0:1, 1:2])[:])
                omega_bc = bcast(omega[:])
                nomega_bc = bcast(s_neg(omega[:])[:])
                if not last:
                    rho_n = s_sub(s_sub(rho_ap, s_mul(alpha[:], d1_ap)[:])[:],
                                  s_mul(omega[:], dsb[0:1, 2:3])[:])
                    beta = s_mul(
                        s_mul(rho_n[:], s_recip_eps(rho_ap)[:])[:],
                        s_mul(alpha[:], s_recip_eps(omega[:])[:])[:])
                    beta_bc = bcast(beta[:])
                    rho_ap = rho_n[:]

                # ===== C: pure update pass (s,t loaded, no stencil):
                #          x += alpha p + omega s; r = s - omega t;
                #          p = r + beta (p - omega v) =====================
                xsrc = x_in if first else xw
                xdst = x_out if last else xw
                order = list(range(NB)) if last else ghost_first
                for oi, j in enumerate(order):
                    for wc in range(2):
                        sc_ = load_blk(fs, wc, j, f"sb{wc}", SD)[:]
                        xt = load_blk(xsrc, wc, j, f"xb{wc}", F32)
                        if first:
                            pc = load_blk(r0f, wc, j, f"pbF{wc}", F32,
                                          bufs=1)[:]
                        else:
                            pc = load_blk(fp, wc, j, f"pb{wc}", SD)[:]
                        x1 = scr_tile(wc)
                        stt(nc.vector, x1[:], pc, alpha_bc[:], xt[:])
                        x2 = scr_tile(wc)
                        stt(nc.vector, x2[:], sc_, omega_bc[:], x1[:])
                        store_blk(xdst, x2, wc, j)
                        if not last:
                            tc_ = load_blk(ft, wc, j, f"sd{wc}", SD)[:]
                            vc = load_blk(fv, wc, j, f"t1{wc}", SD)[:]
                            rt = sb.tile([128, KH, Z], SD, tag=f"q{wc}",
                                         bufs=4, name=_nm("rt"))
                            stt(nc.vector, rt[:], tc_, nomega_bc[:], sc_)
                            store_blk(fr, rt, wc, j)
                            u = sb.tile([128, KH, Z], SD, tag=f"q{wc}",
                                        bufs=4, name=_nm("u"))
                            stt(nc.vector, u[:], vc, nomega_bc[:], pc)
                            po_ = sb.tile([128, KH, Z], SD, tag=f"q{wc}",
                                          bufs=4, name=_nm("po"))
                            stt(nc.vector, po_[:], u[:], beta_bc[:], rt[:])
                            store_blk(fp, po_, wc, j)
                            if j == 0:
                                stage_plane(po_[:, 0, :], 0, wc, halo_inS)
                            if j == NB - 1:
                                stage_plane(po_[:, KH - 1, :], 1, wc,
                                            halo_inS)
                    if not last and oi == 1:
                        allgather(halo_inS, halo_outS)

            _loop.close()
            if twin:
                nc.sync.dma_start(out=dummy_out[:, :], in_=z8[:])

    nc.compile()
    return nc


# ---------------------------------------------------------------------------
# host-side wrapper
# ---------------------------------------------------------------------------
_CACHE = {}


def _shift_mats():
    A = np.zeros((128, 128), np.float32)
    for i in range(127):
        A[i, i + 1] = 1.0
        A[i + 1, i] = 1.0
    B01 = np.zeros((128, 128), np.float32)
    B01[0, 127] = 1.0
    B10 = np.zeros((128, 128), np.float32)
    B10[127, 0] = 1.0
    return np.concatenate([A, B01, B10], axis=1)


def make_in_maps(x, b, center, HC, W, Z, bf16=True):
    import ml_dtypes
    BF = ml_dtypes.bfloat16 if bf16 else np.float32
    mats = _shift_mats()
    matsS = mats.astype(BF)
    ZR = GROUP * 2 * W
    in_maps = []
    for c in range(N_CORES):
        bi, s = divmod(c, GROUP)
        h0 = s * HC
        # [HC, W, Z] -> [W, HC, Z] -> [2][128, HC*Z]
        xc = np.ascontiguousarray(
            x[bi, h0:h0 + HC].transpose(1, 0, 2)).reshape(2, 128, HC * Z)
        bc = np.ascontiguousarray(
            b[bi, h0:h0 + HC].transpose(1, 0, 2)).reshape(2, 128, HC * Z)
        cen = np.ascontiguousarray(
            center[0, h0:h0 + HC, :, 0].astype(np.float32).T).reshape(
                2, 128, HC)
        w = np.arange(W, dtype=np.int32)
        lo = (s - 1) * 2 * W + W + w if s > 0 else ZR + (w % 128)
        hi = (s + 1) * 2 * W + w if s < GROUP - 1 else ZR + (w % 128)
        idx = np.stack([lo, hi], axis=1).astype(np.int32).reshape(2, 128, 2)
        in_maps.append({
            "xc0": xc[0], "xc1": xc[1],
            "bc0": bc[0], "bc1": bc[1],
            "cen": cen, "matsF": mats, "matsS": matsS, "idx": idx,
        })
    return in_maps


RUN_WALL_S = []


def kernel(x, b, ref, center):
    """Full inputs in, full output out. ref is unused by the reference."""
    import time as _time
    B, H, W, Z = x.shape
    HC = H // GROUP
    key = (HC, W, Z)
    if key not in _CACHE:
        _CACHE[key] = build_program(HC=HC, W=W, Z=Z)
    nc = _CACHE[key]

    from concourse.bass_utils import run_bass_kernel_spmd
    in_maps = make_in_maps(np.asarray(x), np.asarray(b), np.asarray(center),
                           HC, W, Z)
    _t0 = _time.time()
    res = run_bass_kernel_spmd(nc, in_maps, core_ids=list(range(N_CORES)))
    RUN_WALL_S.append(_time.time() - _t0)
    out = np.empty((B, H, W, Z), np.float32)
    for c in range(N_CORES):
        bi, s = divmod(c, GROUP)
        xo = np.concatenate([res.results[c]["xo0"], res.results[c]["xo1"]],
                            axis=0)  # [256, HC*Z]
        out[bi, s * HC:(s + 1) * HC] = xo.reshape(
            W, HC, Z).transpose(1, 0, 2)
    return out
